# revision 1
# baseline (speedup 1.0000x reference)
"""Binary-weight 3x3 conv (depth-1 conv3d), 32ch -> 32ch, on trn2.

Forward pass of a BNN conv: effective weights are scale[o,i] * sign(w[o,i,kh,kw])
(the straight-through-estimator machinery in the reference only affects grads).
Kernel depth is 1, so this is a 2D 3x3 same-padded conv applied independently to
each of N*D = 8*16 = 128 images of shape [32, 160, 160].

Strategy (per core; batch dim sharded 1:1 onto 8 cores), variant "f16":
  - x, w and out travel as fp16 on the wire: the host casts x during the pad
    (input HBM traffic halves), the PE runs fp16 matmuls at 1 cycle/row with
    exact products into the f32 PSUM accumulator, the PSUM->SBUF evacuation
    casts to fp16 (output traffic halves), and the host upcasts the result.
    Total rounding error ~3*2^-11 vs the 2e-2 gate.
  - 16 d-slices per core, processed in 4 groups of 4 images.
  - Images live in SBUF zero-padded to 162 cols, on 32 channels = partitions
    [32r, 32r+32) for image r of the group.
  - PE runs in 32x32 tile-packing mode: tile (r, c) computes image r,
    pixel-segment c. 16 concurrent matmuls per tap, 9 taps accumulate in PSUM
    (tap shifts = free-axis offsets into the padded image); the 4 row-tiles
    of a column group share XBUS beats so a tap streams in ~N cycles.
  - H=160 is covered by 13 12-row rounds plus one exact 4-row remainder
    round (seg_rows=1) -- no row is computed twice.
  - PSUM evacuated to SBUF split between DVE and ACT; per-seg output DMAs
    alternate across the two HWDGE rings while input strip DMAs go through
    the gpsimd SWDGE queues so they never FIFO-block the outputs.
  - Measured on hw: 8064 matmuls + ldweights at ~546ns/tap cadence; the
    tap overhead (sem incs + weight reloads), not stream time or DMA,
    is the binding resource at ~331us.
"""

import numpy as np

import concourse.bass as bass
from concourse.bass import InstructionNameOrderedSet
import concourse.mybir as mybir
import concourse.tile as tile
from concourse import bacc
from concourse import bass_utils

C = 32          # in = out channels
KH = KW = 3


def _matmul_noload(nc, out, lhsT, rhs, tile_position, start, stop):
    """InstMatmult with ldweights=False: uses whatever weights the preceding
    explicit LDWEIGHTS left in this PE tile instead of self-loading. (The
    normal fused path is split into InstLdweights+InstMatmult by a bass
    lowering pass for non-f32 ifmaps; ldweights=False marks the matmul as
    already-non-self-loading.) lhsT stays in `ins` for dep tracking only."""
    eng = nc.tensor
    ifmap_ap = eng.lower_ap(rhs.opt(frozenset({0})), opt=False)
    weights_ap = eng.lower_ap(
        lhsT.opt(frozenset({0})), opt=False, for_matmul_weights=True
    )
    out_ap = eng.lower_ap(out)
    return eng.add_instruction(
        mybir.InstMatmult(
            name=eng.bass.get_next_instruction_name(),
            replication_resolution=0,
            replication_shift_amnt=0,
            replication_num_rows=0,
            start_tensor_calc=start,
            stop_tensor_calc=stop,
            ins=[ifmap_ap, weights_ap],
            outs=[out_ap],
            perf_mode=None,
            is_transpose=None,
            ifmap_quant_offset=None,
            weights_quant_offset=None,
            bass_skip_group_check=True,
            tile_position=tile_position,
            tile_size=(32, 32),
            ldweights=True,
        )
    )

# full-problem dims
FULL_N, FULL_D, FULL_H, FULL_W = 8, 16, 160, 160


def _round_list(H, seg_rows, SEGS=4):
    """(origin, rows-per-seg) rounds covering H exactly."""
    RPR = SEGS * seg_rows
    rounds = [(j0, seg_rows) for j0 in range(0, H - RPR + 1, RPR)]
    covered = rounds[-1][0] + RPR
    if covered < H:
        assert (H - covered) % SEGS == 0
        rounds.append((covered, (H - covered) // SEGS))
    return rounds


def build_conv(tc, out_ap, x_ap, w_ap, D, H, W, seg_rows, strip_rows, variant):
    """Emit the conv program for one core. x: [32, D, H, W], out: [32, D, H, W].

    variant "f32":   exact fp32 matmuls (4 cyc/row), w: [128, 288] f32.
    variant "bf16x3": x and w each split into bf16 hi+lo; accumulate
        w_hi*x_hi + w_hi*x_lo + w_lo*x_hi (error ~2^-18), w: [128, 2, 288] bf16.
    """
    nc = tc.nc
    f32 = mybir.dt.float32
    bf16 = mybir.dt.bfloat16
    f16 = mybir.dt.float16
    # f16: x/w/out are cast to fp16 on the host, halving HBM traffic in both
    # directions, and the PE runs 1 cycle/row (vs 4 for exact f32). Products
    # are exact in the f32 PSUM accumulator; the only error is the three
    # 2^-11 roundings (x, w, out) -- ~1e-3 vs the 2e-2 gate.
    # (f32r also hits 1 cyc/row but its matmuls fail the s3d3 dst-partition
    # ISA check under 32x32 column tiling, so it cannot be compiled here.)
    x_dt = {
        "f32r": mybir.dt.float32r, "f16": f16, "f16w": f16, "f16r1": f16,
    }.get(variant, f32)
    st_dt = f16 if variant in ("f16", "f16w", "f16r1") else f32

    IPG = 4                      # images per group (row tiles)
    SEGS = 4                     # pixel segments per round (col tiles)
    NMM = seg_rows * W           # moving free size per matmul
    RPR = SEGS * seg_rows        # output rows per round
    assert D % IPG == 0
    NGRP = D // IPG
    WP = W + 2
    assert NMM <= 512

    # rounds as (origin row, rows-per-seg). If RPR doesn't divide H, the
    # remainder is an exact short round (SEGS segs of H%RPR//SEGS rows each)
    # so no row is ever computed twice.
    rounds = _round_list(H, seg_rows, SEGS)
    rounds_per_strip = max(1, strip_rows // RPR)
    strips = [
        rounds[i : i + rounds_per_strip]
        for i in range(0, len(rounds), rounds_per_strip)
    ]
    rspan = lambda rs: rs[-1][0] + SEGS * rs[-1][1] + 2 - rs[0][0]
    XROWS = max(rspan(rs) for rs in strips)

    # per col group, names of the previous tap's matmuls (f16w ordering chain)
    last_mms = [[] for _ in range(SEGS)]
    # x_ap is host-prepadded: [D, C, H+2, W+2] with zero borders, so a strip
    # is one fully-contiguous DMA per partition (partition stride = (H+2)*(W+2)).
    x_r = x_ap.rearrange("(g p) hp wp -> g p (hp wp)", g=NGRP, p=IPG * C)
    # [g] -> (o, r, h*w): row-sliced per round/seg at DMA time. o outermost
    # so the DGE splits each output DMA across all 16 SDMA engines (it
    # splits on the outermost dest dim; with 4 outermost it used only 4)
    if variant == "f16r1":
        out_v = None          # raw [NGRP, NR, 128, IPG, 480] dump layout
    else:
        out_v = out_ap.rearrange("o (g r) h w -> g o r (h w)", g=NGRP, r=IPG)

    xbytes = XROWS * WP * mybir.dt.size(x_dt)
    xbufs = 3 if (variant != "bf16x3" and 3 * xbytes < 160 * 1024) else 2
    with (
        tc.tile_pool(name="wpool", bufs=1) as wpool,
        tc.tile_pool(name="xpool", bufs=xbufs) as xpool,
        tc.tile_pool(name="stpool", bufs=3) as stpool,
        tc.tile_pool(name="pspool", bufs=2, space="PSUM") as pspool,
    ):
        if variant in ("f32", "f32r", "f16", "f16w", "f16r1"):
            w_sb = wpool.tile([128, KH * KW * C], x_dt, tag="w")
        else:
            w_sb = wpool.tile([128, 2, KH * KW * C], bf16, tag="w")
        nc.sync.dma_start(w_sb[:], w_ap[:])

        for g in range(NGRP):
            for si, strip in enumerate(strips):
                X32 = xpool.tile([128, XROWS, WP], x_dt, tag="X32")
                r0 = strip[0][0]
                nrows = rspan(strip)
                # padded rows [r0, r0+nrows) of each image, contiguous runs
                # per partition. The very first strip is split into per-round
                # chunks so round k is gated only on its own rows, not the
                # whole strip. Inputs go through the gpsimd SWDGE queues so
                # the big strip transfers never FIFO-block the output DMAs
                # on the two HWDGE rings.
                if g == 0 and si == 0 and nrows > 3 * RPR:
                    cuts = list(range(RPR + 2, nrows, RPR)) + [nrows]
                    cuts = [0] + [c for c in cuts if c <= nrows]
                else:
                    cuts = [0, nrows]
                # (measured: routing the first strip's chunks via the sync
                # HWDGE ring instead regresses ~9us — they collide with the
                # early output DMAs; SWDGE everywhere wins)
                # (measured: first-chunk-on-sync + stpool=4 cost +2us
                # vs this configuration; all-SWDGE inputs + 3 st bufs win)
                for a, b in zip(cuts, cuts[1:]):
                    if b <= a:
                        continue
                    nc.gpsimd.dma_start(
                        X32[:, a:b, :].rearrange("p a b -> p (a b)"),
                        x_r[g][:, (r0 + a) * WP : (r0 + b) * WP],
                    )

                if variant in ("f32", "f32r", "f16", "f16w", "f16r1"):
                    # comp -> (weight slice index or None, moving buffer)
                    comps = [(None, X32)]
                else:
                    Xhi = xpool.tile([128, XROWS, WP], bf16, tag="Xhi")
                    Xlo = xpool.tile([128, XROWS, WP], bf16, tag="Xlo")
                    nc.scalar.copy(Xhi[:, 0:nrows, :], X32[:, 0:nrows, :])
                    nc.vector.tensor_sub(
                        Xlo[:, 0:nrows, :], X32[:, 0:nrows, :],
                        Xhi[:, 0:nrows, :],
                    )
                    comps = [(0, Xhi), (0, Xlo), (1, Xhi)]

                ri0 = si * rounds_per_strip
                for rk, (j0, sr) in enumerate(strip):
                    ri = ri0 + rk
                    nmm = sr * W
                    ps = pspool.tile([128, SEGS, 512], f32, tag="ps")
                    for ci, (wi, XB) in enumerate(comps):
                        for tap in range(KH * KW):
                            kh, kw = divmod(tap, KW)
                            if variant == "f16w":
                                # one [128,32] LDWEIGHTS per col group fills
                                # all 4 row tiles at once (w_sb partitions are
                                # 4 replicas of the 32-ch weight block); the
                                # 16 matmuls then run without self-loading.
                                # nosync deps pin the per-engine stream order
                                # the IR no longer expresses.
                                lds = []
                                for c in range(SEGS):
                                    ld = nc.tensor.ldweights(
                                        w_sb[:, 32 * tap : 32 * tap + 32],
                                        tile_position=(0, 32 * c),
                                    )
                                    if last_mms[c]:
                                        ld.ins.add_nosync_dependencies_from(
                                            InstructionNameOrderedSet(
                                                last_mms[c]
                                            )
                                        )
                                    lds.append(ld)
                                for c in range(SEGS):
                                    last_mms[c] = []
                                for c in range(SEGS):
                                    for r in range(IPG):
                                        j = j0 - r0 + sr * c
                                        rhs = XB[
                                            32 * r : 32 * r + 32,
                                            j + kh : j + kh + sr,
                                            kw : kw + W,
                                        ]
                                        mm = _matmul_noload(
                                            nc,
                                            ps[32 * c : 32 * c + 32, r, 0:nmm],
                                            w_sb[
                                                32 * r : 32 * r + 32,
                                                32 * tap : 32 * tap + 32,
                                            ],
                                            rhs,
                                            (32 * r, 32 * c),
                                            start=(ci == 0 and tap == 0),
                                            stop=(
                                                ci == len(comps) - 1
                                                and tap == KH * KW - 1
                                            ),
                                        )
                                        mm.ins.add_nosync_dependencies_from(
                                            InstructionNameOrderedSet(
                                                [lds[c].ins.name]
                                            )
                                        )
                                        last_mms[c].append(mm.ins.name)
                                continue
                            # c innermost: consecutive matmuls (and their
                            # legalization-inserted weight loads) hit
                            # different PE COLUMN groups; XBUSes are wired
                            # per column group, so this is what lets the
                            # loads run concurrently instead of serializing
                            # on one column's bus.
                            for r in range(IPG):
                                for c in range(SEGS):
                                    if wi is None:
                                        lhsT = w_sb[
                                            32 * r : 32 * r + 32,
                                            32 * tap : 32 * tap + 32,
                                        ]
                                    else:
                                        lhsT = w_sb[
                                            32 * r : 32 * r + 32, wi,
                                            32 * tap : 32 * tap + 32,
                                        ]
                                    j = j0 - r0 + sr * c
                                    rhs = XB[
                                        32 * r : 32 * r + 32,
                                        j + kh : j + kh + sr,
                                        kw : kw + W,
                                    ]
                                    nc.tensor.matmul(
                                        ps[32 * c : 32 * c + 32, r, 0:nmm],
                                        lhsT,
                                        rhs,
                                        start=(ci == 0 and tap == 0),
                                        stop=(
                                            ci == len(comps) - 1
                                            and tap == KH * KW - 1
                                        ),
                                        tile_position=(32 * r, 32 * c),
                                    )
                    st = stpool.tile([128, SEGS, nmm], st_dt, tag="st")
                    nc.vector.tensor_copy(st[:, 0:2, :], ps[:, 0:2, 0:nmm])
                    nc.scalar.copy(st[:, 2:4, :], ps[:, 2:4, 0:nmm])
                    if variant == "f16r1":
                        # ONE dma per round into the raw dump layout
                        # [g, ri, 128, r, q]; the host unscrambles. Quarters
                        # HWDGE ring occupancy vs 4 per-seg DMAs.
                        eng = nc.sync if ri % 2 == 0 else nc.scalar
                        eng.dma_start(
                            out_ap[g, ri, :, :, 0:nmm],
                            st[:, :, :],
                        )
                        continue
                    for c in range(SEGS):
                        eng = nc.sync if c % 2 == 0 else nc.scalar
                        lo_px = (j0 + sr * c) * W
                        eng.dma_start(
                            out_v[g][:, :, lo_px : lo_px + nmm],
                            st[32 * c : 32 * c + 32, :, :],
                        )


def build_module(n_cores=8, D=FULL_D, H=FULL_H, W=FULL_W, seg_rows=3,
                 strip_rows=None, variant="f32"):
    if strip_rows is None:
        strip_rows = 36 if variant == "bf16x3" else 96
    nc = bacc.Bacc(
        "TRN2",
        target_bir_lowering=False,
        debug=False,
        num_devices=n_cores,
    )
    in_dt = {
        "f32r": mybir.dt.float32r, "f16": mybir.dt.float16,
        "f16w": mybir.dt.float16, "f16r1": mybir.dt.float16,
    }.get(variant, mybir.dt.float32)
    out_dt = (
        mybir.dt.float16 if variant in ("f16", "f16w", "f16r1")
        else mybir.dt.float32
    )
    x_d = nc.dram_tensor(
        "x", [D * C, H + 2, W + 2], in_dt, kind="ExternalInput"
    )
    if variant in ("f32", "f32r", "f16", "f16w", "f16r1"):
        w_d = nc.dram_tensor(
            "w", [128, KH * KW * C], in_dt, kind="ExternalInput"
        )
    else:
        w_d = nc.dram_tensor(
            "w", [128, 2, KH * KW * C], mybir.dt.bfloat16, kind="ExternalInput"
        )
    if variant == "f16r1":
        nr = len(_round_list(H, seg_rows))
        out_d = nc.dram_tensor(
            "out", [D // 4, nr, 128, 4, 3 * W], out_dt, kind="ExternalOutput"
        )
    else:
        out_d = nc.dram_tensor(
            "out", [C, D, H, W], out_dt, kind="ExternalOutput"
        )
    with tile.TileContext(nc) as tc:
        build_conv(
            tc, out_d.ap(), x_d.ap(), w_d.ap(), D, H, W, seg_rows, strip_rows,
            variant,
        )
    nc.compile()
    return nc


def binarize_weights(weights, variant="bf16x3"):
    """Host-side: [32,32,1,3,3] fp32 -> packed replicated weight tile.
    w_packed[32r+i, 32*tap+o] = scale[o,i] * sign(w[o,i,kh,kw]), tap = kh*3+kw.
    f32: [128, 288] f32.  bf16x3: [128, 2, 288] bf16 (hi, lo split)."""
    w = np.asarray(weights, dtype=np.float32)
    scale = np.mean(np.abs(w), axis=(2, 3, 4), keepdims=True)
    bw = (scale * np.sign(w)).astype(np.float32)          # [o, i, 1, 3, 3]
    wt = bw[:, :, 0].transpose(1, 2, 3, 0).reshape(C, KH * KW * C)  # [i, tap*32+o]
    full = np.ascontiguousarray(np.tile(wt, (4, 1)))       # [128, 288] f32
    if variant in ("f32", "f32r"):
        return full
    if variant in ("f16", "f16w", "f16r1"):
        return full.astype(np.float16)
    import ml_dtypes
    hi = full.astype(ml_dtypes.bfloat16)
    lo = (full - hi.astype(np.float32)).astype(ml_dtypes.bfloat16)
    return np.ascontiguousarray(np.stack([hi, lo], axis=1))  # [128, 2, 288] bf16


_NC_CACHE = {}


def _get_nc(key, **kwargs):
    if key not in _NC_CACHE:
        _NC_CACHE[key] = build_module(**kwargs)
    return _NC_CACHE[key]


def pad_input(x, np_dt=np.float32):
    """[N, C, D, H, W] f32 -> [N, D*C, H+2, W+2] zero-padded, d-major."""
    n, c, d, h, w = x.shape
    xp = np.zeros((n, d, c, h + 2, w + 2), dtype=np_dt)
    xp[:, :, :, 1 : h + 1, 1 : w + 1] = x.transpose(0, 2, 1, 3, 4)
    return xp.reshape(n, d * c, h + 2, w + 2)


def run(x, weights, trace=False, variant="f16r1", seg_rows=3, strip_rows=None):
    x = np.asarray(x, dtype=np.float32)
    n_cores = x.shape[0]
    key = (n_cores, variant, seg_rows, strip_rows)
    nc = _get_nc(
        key, n_cores=n_cores, seg_rows=seg_rows, strip_rows=strip_rows,
        variant=variant,
    )
    xp = pad_input(
        x, np.float16 if variant in ("f16", "f16w", "f16r1") else np.float32
    )
    w_packed = binarize_weights(weights, variant)
    in_maps = [{"x": xp[n], "w": w_packed} for n in range(n_cores)]
    res = bass_utils.run_bass_kernel_spmd(
        nc, in_maps, core_ids=list(range(n_cores)), trace=trace
    )
    out = np.stack([res.results[n]["out"] for n in range(n_cores)])
    if variant == "f16r1":
        out = _unscramble(out)
    if out.dtype != np.float32:
        out = out.astype(np.float32)
    return out, res


def _unscramble(raw, H=FULL_H, W=FULL_W, seg_rows=3):
    """[n, NGRP, NR, 128, 4, 3W] f16 raw dump -> [n, C, D, H, W] f32.
    raw[n, g, ri, 32c+o, r, u*W+v] = out[n, o, 4g+r, j0(ri)+sr*c+u, v]."""
    n, ngrp, nr = raw.shape[:3]
    rounds = _round_list(H, seg_rows)
    out = np.empty((n, C, ngrp * 4, H, W), dtype=np.float32)
    full = [k for k, (_, sr) in enumerate(rounds) if sr == seg_rows]
    assert full == list(range(len(full)))
    nf = len(full)
    f = raw[:, :, :nf].reshape(n, ngrp, nf, 4, C, 4, seg_rows, W)
    # [n, g, ri, c, o, r, u, v] -> [n, o, g, r, ri, c, u, v]
    out[:, :, :, : nf * 4 * seg_rows, :] = (
        f.transpose(0, 4, 1, 5, 2, 3, 6, 7)
        .reshape(n, C, ngrp * 4, nf * 4 * seg_rows, W)
    )
    for k in range(nf, nr):
        j0, sr = rounds[k]
        s = raw[:, :, k, :, :, : sr * W].reshape(
            n, ngrp, 4, C, 4, sr, W
        )
        out[:, :, :, j0 : j0 + 4 * sr, :] = (
            s.transpose(0, 3, 1, 4, 2, 5, 6)
            .reshape(n, C, ngrp * 4, 4 * sr, W)
        )
    return out


def kernel(x, weights):
    out, _ = run(x, weights)
    return out



# revision 3
# speedup vs baseline: 1.1532x; 1.1532x over previous
"""Binary-weight 3x3 conv (depth-1 conv3d), 32ch -> 32ch, on trn2.

Forward pass of a BNN conv: effective weights are scale[o,i] * sign(w[o,i,kh,kw])
(the straight-through-estimator machinery in the reference only affects grads).
Kernel depth is 1, so this is a 2D 3x3 same-padded conv applied independently to
each of N*D = 8*16 = 128 images of shape [32, 160, 160].

Strategy (per core; batch dim sharded 1:1 onto 8 cores), variant "f16":
  - x, w and out travel as fp16 on the wire: the host casts x during the pad
    (input HBM traffic halves), the PE runs fp16 matmuls at 1 cycle/row with
    exact products into the f32 PSUM accumulator, the PSUM->SBUF evacuation
    casts to fp16 (output traffic halves), and the host upcasts the result.
    Total rounding error ~3*2^-11 vs the 2e-2 gate.
  - 16 d-slices per core, processed in 4 groups of 4 images.
  - Images live in SBUF zero-padded to 162 cols, on 32 channels = partitions
    [32r, 32r+32) for image r of the group.
  - PE runs in 32x32 tile-packing mode: tile (r, c) computes image r,
    pixel-segment c. 16 concurrent matmuls per tap, 9 taps accumulate in PSUM
    (tap shifts = free-axis offsets into the padded image); the 4 row-tiles
    of a column group share XBUS beats so a tap streams in ~N cycles.
  - H=160 is covered by 13 12-row rounds plus one exact 4-row remainder
    round (seg_rows=1) -- no row is computed twice.
  - PSUM evacuated to SBUF split between DVE and ACT; per-seg output DMAs
    alternate across the two HWDGE rings while input strip DMAs go through
    the gpsimd SWDGE queues so they never FIFO-block the outputs.
  - Measured on hw: 8064 matmuls + ldweights at ~546ns/tap cadence; the
    tap overhead (sem incs + weight reloads), not stream time or DMA,
    is the binding resource at ~331us.
"""

import numpy as np

import concourse.bass as bass
from concourse.bass import InstructionNameOrderedSet
import concourse.mybir as mybir
import concourse.tile as tile
from concourse import bacc
from concourse import bass_utils

C = 32          # in = out channels
KH = KW = 3


def _matmul_noload(nc, out, lhsT, rhs, tile_position, start, stop):
    """InstMatmult with ldweights=False: uses whatever weights the preceding
    explicit LDWEIGHTS left in this PE tile instead of self-loading. (The
    normal fused path is split into InstLdweights+InstMatmult by a bass
    lowering pass for non-f32 ifmaps; ldweights=False marks the matmul as
    already-non-self-loading.) lhsT stays in `ins` for dep tracking only."""
    eng = nc.tensor
    ifmap_ap = eng.lower_ap(rhs.opt(frozenset({0})), opt=False)
    weights_ap = eng.lower_ap(
        lhsT.opt(frozenset({0})), opt=False, for_matmul_weights=True
    )
    out_ap = eng.lower_ap(out)
    return eng.add_instruction(
        mybir.InstMatmult(
            name=eng.bass.get_next_instruction_name(),
            replication_resolution=0,
            replication_shift_amnt=0,
            replication_num_rows=0,
            start_tensor_calc=start,
            stop_tensor_calc=stop,
            ins=[ifmap_ap, weights_ap],
            outs=[out_ap],
            perf_mode=None,
            is_transpose=None,
            ifmap_quant_offset=None,
            weights_quant_offset=None,
            bass_skip_group_check=True,
            tile_position=tile_position,
            tile_size=(32, 32),
            ldweights=True,
        )
    )

# full-problem dims
FULL_N, FULL_D, FULL_H, FULL_W = 8, 16, 160, 160


def _demote_pe_sync(nc, ins):
    """Demote this instruction's same-engine (PE) sync deps to nosync edges.

    Tile's vector-clock sync makes every instruction with sync descendants
    tick a counting semaphore at completion; on hw each tick costs the PE
    sequencer ~34ns of issue bandwidth, which at 16 matmuls/tap pins the tap
    cadence to ~547ns (the measured baseline bottleneck; stream time is only
    ~200ns). PE->PE edges (PSUM accumulate chains) are enforced for free by
    per-tile in-order execution, so carrying them as nosync (scheduler
    ordering only) drops the dead ticks; cross-engine edges (DMA->mm,
    evac->mm WAR) keep their semaphores. Evac's own sync deps still point at
    the stop matmuls, so those (16/round) keep ticking -- which is exactly
    the set whose completion other engines truly need."""
    sync = ins.take_sync_dependencies()
    keep = InstructionNameOrderedSet()
    demote = InstructionNameOrderedSet()
    for name in sync:
        dep = nc.inst_map.get(name) if hasattr(nc.inst_map, "get") else nc.inst_map[name]
        if dep is not None and dep.engine == mybir.EngineType.PE:
            demote.add(name)
        else:
            keep.add(name)
    ins.set_sync_dependencies(keep)
    if demote:
        ins.add_nosync_dependencies_from(demote)


def _round_list(H, seg_rows, SEGS=4):
    """(origin, rows-per-seg) rounds covering H exactly."""
    RPR = SEGS * seg_rows
    rounds = [(j0, seg_rows) for j0 in range(0, H - RPR + 1, RPR)]
    covered = rounds[-1][0] + RPR
    if covered < H:
        assert (H - covered) % SEGS == 0
        rounds.append((covered, (H - covered) // SEGS))
    return rounds


def build_conv(tc, out_ap, x_ap, w_ap, D, H, W, seg_rows, strip_rows, variant):
    """Emit the conv program for one core. x: [32, D, H, W], out: [32, D, H, W].

    variant "f32":   exact fp32 matmuls (4 cyc/row), w: [128, 288] f32.
    variant "bf16x3": x and w each split into bf16 hi+lo; accumulate
        w_hi*x_hi + w_hi*x_lo + w_lo*x_hi (error ~2^-18), w: [128, 2, 288] bf16.
    """
    nc = tc.nc
    f32 = mybir.dt.float32
    bf16 = mybir.dt.bfloat16
    f16 = mybir.dt.float16
    # f16: x/w/out are cast to fp16 on the host, halving HBM traffic in both
    # directions, and the PE runs 1 cycle/row (vs 4 for exact f32). Products
    # are exact in the f32 PSUM accumulator; the only error is the three
    # 2^-11 roundings (x, w, out) -- ~1e-3 vs the 2e-2 gate.
    # (f32r also hits 1 cyc/row but its matmuls fail the s3d3 dst-partition
    # ISA check under 32x32 column tiling, so it cannot be compiled here.)
    x_dt = {
        "f32r": mybir.dt.float32r, "f16": f16, "f16w": f16, "f16r1": f16,
        "f16d": f16,
    }.get(variant, f32)
    st_dt = f16 if variant in ("f16", "f16w", "f16r1", "f16d") else f32

    IPG = 4                      # images per group (row tiles)
    SEGS = 4                     # pixel segments per round (col tiles)
    NMM = seg_rows * W           # moving free size per matmul
    RPR = SEGS * seg_rows        # output rows per round
    assert D % IPG == 0
    NGRP = D // IPG
    WP = W + 2
    assert NMM <= 512

    # rounds as (origin row, rows-per-seg). If RPR doesn't divide H, the
    # remainder is an exact short round (SEGS segs of H%RPR//SEGS rows each)
    # so no row is ever computed twice.
    rounds = _round_list(H, seg_rows, SEGS)
    rounds_per_strip = max(1, strip_rows // RPR)
    strips = [
        rounds[i : i + rounds_per_strip]
        for i in range(0, len(rounds), rounds_per_strip)
    ]
    rspan = lambda rs: rs[-1][0] + SEGS * rs[-1][1] + 2 - rs[0][0]
    XROWS = max(rspan(rs) for rs in strips)

    # per col group, names of the previous tap's matmuls (f16w ordering chain)
    last_mms = [[] for _ in range(SEGS)]
    # x_ap is host-prepadded: [D, C, H+2, W+2] with zero borders, so a strip
    # is one fully-contiguous DMA per partition (partition stride = (H+2)*(W+2)).
    x_r = x_ap.rearrange("(g p) hp wp -> g p (hp wp)", g=NGRP, p=IPG * C)
    # [g] -> (o, r, h*w): row-sliced per round/seg at DMA time. o outermost
    # so the DGE splits each output DMA across all 16 SDMA engines (it
    # splits on the outermost dest dim; with 4 outermost it used only 4)
    if variant in ("f16r1", "f16d"):
        out_v = None          # raw [NGRP, NR, 128, IPG, 480] dump layout
    else:
        out_v = out_ap.rearrange("o (g r) h w -> g o r (h w)", g=NGRP, r=IPG)

    xbytes = XROWS * WP * mybir.dt.size(x_dt)
    xbufs = 3 if (variant != "bf16x3" and 3 * xbytes < 160 * 1024) else 2
    with (
        tc.tile_pool(name="wpool", bufs=1) as wpool,
        tc.tile_pool(name="xpool", bufs=xbufs) as xpool,
        tc.tile_pool(name="stpool", bufs=3) as stpool,
        tc.tile_pool(name="pspool", bufs=2, space="PSUM") as pspool,
    ):
        if variant in ("f32", "f32r", "f16", "f16w", "f16r1", "f16d"):
            w_sb = wpool.tile([128, KH * KW * C], x_dt, tag="w")
        else:
            w_sb = wpool.tile([128, 2, KH * KW * C], bf16, tag="w")
        nc.sync.dma_start(w_sb[:], w_ap[:])

        for g in range(NGRP):
            for si, strip in enumerate(strips):
                X32 = xpool.tile([128, XROWS, WP], x_dt, tag="X32")
                r0 = strip[0][0]
                nrows = rspan(strip)
                # padded rows [r0, r0+nrows) of each image, contiguous runs
                # per partition. The very first strip is split into per-round
                # chunks so round k is gated only on its own rows, not the
                # whole strip. Inputs go through the gpsimd SWDGE queues so
                # the big strip transfers never FIFO-block the output DMAs
                # on the two HWDGE rings.
                if g == 0 and si == 0 and nrows > 3 * RPR:
                    cuts = list(range(RPR + 2, nrows, RPR)) + [nrows]
                    cuts = [0] + [c for c in cuts if c <= nrows]
                else:
                    cuts = [0, nrows]
                # (measured: routing the first strip's chunks via the sync
                # HWDGE ring instead regresses ~9us — they collide with the
                # early output DMAs; SWDGE everywhere wins)
                # (measured: first-chunk-on-sync + stpool=4 cost +2us
                # vs this configuration; all-SWDGE inputs + 3 st bufs win)
                for a, b in zip(cuts, cuts[1:]):
                    if b <= a:
                        continue
                    nc.gpsimd.dma_start(
                        X32[:, a:b, :].rearrange("p a b -> p (a b)"),
                        x_r[g][:, (r0 + a) * WP : (r0 + b) * WP],
                    )

                if variant in ("f32", "f32r", "f16", "f16w", "f16r1", "f16d"):
                    # comp -> (weight slice index or None, moving buffer)
                    comps = [(None, X32)]
                else:
                    Xhi = xpool.tile([128, XROWS, WP], bf16, tag="Xhi")
                    Xlo = xpool.tile([128, XROWS, WP], bf16, tag="Xlo")
                    nc.scalar.copy(Xhi[:, 0:nrows, :], X32[:, 0:nrows, :])
                    nc.vector.tensor_sub(
                        Xlo[:, 0:nrows, :], X32[:, 0:nrows, :],
                        Xhi[:, 0:nrows, :],
                    )
                    comps = [(0, Xhi), (0, Xlo), (1, Xhi)]

                ri0 = si * rounds_per_strip
                for rk, (j0, sr) in enumerate(strip):
                    ri = ri0 + rk
                    nmm = sr * W
                    ps = pspool.tile([128, SEGS, 512], f32, tag="ps")
                    for ci, (wi, XB) in enumerate(comps):
                        for tap in range(KH * KW):
                            kh, kw = divmod(tap, KW)
                            if variant == "f16w":
                                # one [128,32] LDWEIGHTS per col group fills
                                # all 4 row tiles at once (w_sb partitions are
                                # 4 replicas of the 32-ch weight block); the
                                # 16 matmuls then run without self-loading.
                                # nosync deps pin the per-engine stream order
                                # the IR no longer expresses.
                                lds = []
                                for c in range(SEGS):
                                    ld = nc.tensor.ldweights(
                                        w_sb[:, 32 * tap : 32 * tap + 32],
                                        tile_position=(0, 32 * c),
                                    )
                                    if last_mms[c]:
                                        ld.ins.add_nosync_dependencies_from(
                                            InstructionNameOrderedSet(
                                                last_mms[c]
                                            )
                                        )
                                    lds.append(ld)
                                for c in range(SEGS):
                                    last_mms[c] = []
                                for c in range(SEGS):
                                    for r in range(IPG):
                                        j = j0 - r0 + sr * c
                                        rhs = XB[
                                            32 * r : 32 * r + 32,
                                            j + kh : j + kh + sr,
                                            kw : kw + W,
                                        ]
                                        mm = _matmul_noload(
                                            nc,
                                            ps[32 * c : 32 * c + 32, r, 0:nmm],
                                            w_sb[
                                                32 * r : 32 * r + 32,
                                                32 * tap : 32 * tap + 32,
                                            ],
                                            rhs,
                                            (32 * r, 32 * c),
                                            start=(ci == 0 and tap == 0),
                                            stop=(
                                                ci == len(comps) - 1
                                                and tap == KH * KW - 1
                                            ),
                                        )
                                        mm.ins.add_nosync_dependencies_from(
                                            InstructionNameOrderedSet(
                                                [lds[c].ins.name]
                                            )
                                        )
                                        last_mms[c].append(mm.ins.name)
                                continue
                            # c innermost: consecutive matmuls (and their
                            # legalization-inserted weight loads) hit
                            # different PE COLUMN groups; XBUSes are wired
                            # per column group, so this is what lets the
                            # loads run concurrently instead of serializing
                            # on one column's bus.
                            for r in range(IPG):
                                for c in range(SEGS):
                                    if wi is None:
                                        lhsT = w_sb[
                                            32 * r : 32 * r + 32,
                                            32 * tap : 32 * tap + 32,
                                        ]
                                    else:
                                        lhsT = w_sb[
                                            32 * r : 32 * r + 32, wi,
                                            32 * tap : 32 * tap + 32,
                                        ]
                                    j = j0 - r0 + sr * c
                                    rhs = XB[
                                        32 * r : 32 * r + 32,
                                        j + kh : j + kh + sr,
                                        kw : kw + W,
                                    ]
                                    mm = nc.tensor.matmul(
                                        ps[32 * c : 32 * c + 32, r, 0:nmm],
                                        lhsT,
                                        rhs,
                                        start=(ci == 0 and tap == 0),
                                        stop=(
                                            ci == len(comps) - 1
                                            and tap == KH * KW - 1
                                        ),
                                        tile_position=(32 * r, 32 * c),
                                    )
                                    if variant == "f16d":
                                        _demote_pe_sync(nc, mm.ins)
                    st = stpool.tile([128, SEGS, nmm], st_dt, tag="st")
                    nc.vector.tensor_copy(st[:, 0:2, :], ps[:, 0:2, 0:nmm])
                    nc.scalar.copy(st[:, 2:4, :], ps[:, 2:4, 0:nmm])
                    if variant in ("f16r1", "f16d"):
                        # ONE dma per round into the raw dump layout
                        # [g, ri, 128, r, q]; the host unscrambles. Quarters
                        # HWDGE ring occupancy vs 4 per-seg DMAs.
                        eng = nc.sync if ri % 2 == 0 else nc.scalar
                        eng.dma_start(
                            out_ap[g, ri, :, :, 0:nmm],
                            st[:, :, :],
                        )
                        continue
                    for c in range(SEGS):
                        eng = nc.sync if c % 2 == 0 else nc.scalar
                        lo_px = (j0 + sr * c) * W
                        eng.dma_start(
                            out_v[g][:, :, lo_px : lo_px + nmm],
                            st[32 * c : 32 * c + 32, :, :],
                        )


def build_module(n_cores=8, D=FULL_D, H=FULL_H, W=FULL_W, seg_rows=3,
                 strip_rows=None, variant="f32"):
    if strip_rows is None:
        strip_rows = 36 if variant == "bf16x3" else 96
    nc = bacc.Bacc(
        "TRN2",
        target_bir_lowering=False,
        debug=False,
        num_devices=n_cores,
    )
    in_dt = {
        "f32r": mybir.dt.float32r, "f16": mybir.dt.float16,
        "f16w": mybir.dt.float16, "f16r1": mybir.dt.float16,
        "f16d": mybir.dt.float16,
    }.get(variant, mybir.dt.float32)
    out_dt = (
        mybir.dt.float16 if variant in ("f16", "f16w", "f16r1", "f16d")
        else mybir.dt.float32
    )
    x_d = nc.dram_tensor(
        "x", [D * C, H + 2, W + 2], in_dt, kind="ExternalInput"
    )
    if variant in ("f32", "f32r", "f16", "f16w", "f16r1", "f16d"):
        w_d = nc.dram_tensor(
            "w", [128, KH * KW * C], in_dt, kind="ExternalInput"
        )
    else:
        w_d = nc.dram_tensor(
            "w", [128, 2, KH * KW * C], mybir.dt.bfloat16, kind="ExternalInput"
        )
    if variant in ("f16r1", "f16d"):
        nr = len(_round_list(H, seg_rows))
        out_d = nc.dram_tensor(
            "out", [D // 4, nr, 128, 4, 3 * W], out_dt, kind="ExternalOutput"
        )
    else:
        out_d = nc.dram_tensor(
            "out", [C, D, H, W], out_dt, kind="ExternalOutput"
        )
    with tile.TileContext(nc) as tc:
        build_conv(
            tc, out_d.ap(), x_d.ap(), w_d.ap(), D, H, W, seg_rows, strip_rows,
            variant,
        )
    nc.compile()
    return nc


def binarize_weights(weights, variant="bf16x3"):
    """Host-side: [32,32,1,3,3] fp32 -> packed replicated weight tile.
    w_packed[32r+i, 32*tap+o] = scale[o,i] * sign(w[o,i,kh,kw]), tap = kh*3+kw.
    f32: [128, 288] f32.  bf16x3: [128, 2, 288] bf16 (hi, lo split)."""
    w = np.asarray(weights, dtype=np.float32)
    scale = np.mean(np.abs(w), axis=(2, 3, 4), keepdims=True)
    bw = (scale * np.sign(w)).astype(np.float32)          # [o, i, 1, 3, 3]
    wt = bw[:, :, 0].transpose(1, 2, 3, 0).reshape(C, KH * KW * C)  # [i, tap*32+o]
    full = np.ascontiguousarray(np.tile(wt, (4, 1)))       # [128, 288] f32
    if variant in ("f32", "f32r"):
        return full
    if variant in ("f16", "f16w", "f16r1", "f16d"):
        return full.astype(np.float16)
    import ml_dtypes
    hi = full.astype(ml_dtypes.bfloat16)
    lo = (full - hi.astype(np.float32)).astype(ml_dtypes.bfloat16)
    return np.ascontiguousarray(np.stack([hi, lo], axis=1))  # [128, 2, 288] bf16


_NC_CACHE = {}


def _get_nc(key, **kwargs):
    if key not in _NC_CACHE:
        _NC_CACHE[key] = build_module(**kwargs)
    return _NC_CACHE[key]


def pad_input(x, np_dt=np.float32):
    """[N, C, D, H, W] f32 -> [N, D*C, H+2, W+2] zero-padded, d-major."""
    n, c, d, h, w = x.shape
    xp = np.zeros((n, d, c, h + 2, w + 2), dtype=np_dt)
    xp[:, :, :, 1 : h + 1, 1 : w + 1] = x.transpose(0, 2, 1, 3, 4)
    return xp.reshape(n, d * c, h + 2, w + 2)


def run(x, weights, trace=False, variant="f16r1", seg_rows=3, strip_rows=None):
    x = np.asarray(x, dtype=np.float32)
    n_cores = x.shape[0]
    key = (n_cores, variant, seg_rows, strip_rows)
    nc = _get_nc(
        key, n_cores=n_cores, seg_rows=seg_rows, strip_rows=strip_rows,
        variant=variant,
    )
    xp = pad_input(
        x, np.float16 if variant in ("f16", "f16w", "f16r1", "f16d") else np.float32
    )
    w_packed = binarize_weights(weights, variant)
    in_maps = [{"x": xp[n], "w": w_packed} for n in range(n_cores)]
    res = bass_utils.run_bass_kernel_spmd(
        nc, in_maps, core_ids=list(range(n_cores)), trace=trace
    )
    out = np.stack([res.results[n]["out"] for n in range(n_cores)])
    if variant in ("f16r1", "f16d"):
        out = _unscramble(out)
    if out.dtype != np.float32:
        out = out.astype(np.float32)
    return out, res


def _unscramble(raw, H=FULL_H, W=FULL_W, seg_rows=3):
    """[n, NGRP, NR, 128, 4, 3W] f16 raw dump -> [n, C, D, H, W] f32.
    raw[n, g, ri, 32c+o, r, u*W+v] = out[n, o, 4g+r, j0(ri)+sr*c+u, v]."""
    n, ngrp, nr = raw.shape[:3]
    rounds = _round_list(H, seg_rows)
    out = np.empty((n, C, ngrp * 4, H, W), dtype=np.float32)
    full = [k for k, (_, sr) in enumerate(rounds) if sr == seg_rows]
    assert full == list(range(len(full)))
    nf = len(full)
    f = raw[:, :, :nf].reshape(n, ngrp, nf, 4, C, 4, seg_rows, W)
    # [n, g, ri, c, o, r, u, v] -> [n, o, g, r, ri, c, u, v]
    out[:, :, :, : nf * 4 * seg_rows, :] = (
        f.transpose(0, 4, 1, 5, 2, 3, 6, 7)
        .reshape(n, C, ngrp * 4, nf * 4 * seg_rows, W)
    )
    for k in range(nf, nr):
        j0, sr = rounds[k]
        s = raw[:, :, k, :, :, : sr * W].reshape(
            n, ngrp, 4, C, 4, sr, W
        )
        out[:, :, :, j0 : j0 + 4 * sr, :] = (
            s.transpose(0, 3, 1, 4, 2, 5, 6)
            .reshape(n, C, ngrp * 4, 4 * sr, W)
        )
    return out


def kernel(x, weights):
    out, _ = run(x, weights)
    return out

